# revision 29
# baseline (speedup 1.0000x reference)
"""Distributed Trainium2 kernel for a single causal attention head.

Problem (hardcoded): B=4, S=2048, D_MODEL=1024, HEAD_DIM=64, fp32 inputs.
    q = query @ Wq + bq ; k = key @ Wk + bk ; v = value @ Wv + bv
    scores = q k^T / sqrt(H) ; masked softmax ; out = att @ v

Sharding (8 NeuronCores): KEY-SPLIT partial softmax.  Core c = (b, h)
with b = c//2, h = c%2.  Each core handles ALL 2048 query rows of its
batch but only HALF of the keys: h=0 owns global key j-tiles
{0,3,4,7,8,11,12,15}, h=1 owns {1,2,5,6,9,10,13,14} (tile = 128 keys).
This interleave makes causal extents identical in LOCAL tile index on
both cores: query pair p (512 rows, chunks 2p,2p+1) attends exactly
local tiles 0..2p+1 (tiles 2p wide-predicated, 2p+1 solo-predicated),
so one SPMD program serves all cores; per-core differences are pure
data (packed k/v halves and predicate thresholds).

Each core computes UNNORMALIZED partials: po[0:64, i] = sum_j att*v,
po[64, i] = sum_j att (the denominator, via an appended ones row in
v_aug).  No on-device normalization: the raw [65, 2048] fp32 partials
are DMAed out and the host combines (numA+numB)/(denA+denB) per batch.
This halves the per-core k/v DMA (10.5 MB -> 8 MB) and removes the
on-device transpose/reciprocal epilogue.

Device layout: query/key/value shards passed TRANSPOSED and pre-packed
([128, D/128, cols] bf16, one contiguous DMA line per partition); all
matmuls contract over the partition dim in natural layout:
  qT/kT/vT[h,:] = W^T X^T  (col-group-paired projection matmuls)
  v[j,h]        = vT via PE-transpose, ones col appended -> row 64
  sT[j,i]       = k-tile as lhsT, rhs=qT   (scores transposed)
  att           = exp(sT * 0.125) (ScalarE, PSUM->SBUF, bf16) * pred
  po[65,i]     += v_aug-tile as lhsT, rhs=att
DMA discipline: big inputs ride ONE HWDGE ring (sync) in dependency
order k0, q(4x1MB), k1, v0, v1a, v1b so the PE is fed continuously;
constants+thresholds go on the scalar ring; per-pair [65,512] outputs
leave via gpsimd SWDGE as soon as each pair's last av retires.
"""

import os

import numpy as np
import ml_dtypes

import concourse.bass as bass
import concourse.tile as tile
from concourse import bacc, mybir
from concourse.bass import ds
from concourse.bass_utils import run_bass_kernel_spmd
from concourse.masks import make_identity

B, S, D, H = 4, 2048, 1024, 64
P = 128
NCORES = 8
CHUNK = 256               # query rows per chunk
NQ = S                    # every core sees all 2048 query rows
SL = S // 2               # local keys per core (1024)
JTL = SL // P             # 8 local j-tiles
NPAIRS = 4                # pairs of 512 query rows
DCH = D // P              # 8 contraction chunks
FP = mybir.dt.float32
BF = mybir.dt.bfloat16
BF_NP = ml_dtypes.bfloat16

# local-tile causal extents per pair: (shared/wide extent, solo extent)
KS_PAIRS = tuple((2 * p + 1, 2 * p + 2) for p in range(NPAIRS))
# predicated (pair, local j-tile) slots: last two tiles of each pair
KS_MASKED = [(p, t) for p in range(NPAIRS) for t in (2 * p, 2 * p + 1)]
# global j-tile pattern per h (balanced causal interleave)
JGLOB = {
    0: [0, 3, 4, 7, 8, 11, 12, 15],
    1: [1, 2, 5, 6, 9, 10, 13, 14],
}

LAST_RESULTS = None
_PROGRAM_CACHE = {}


def _build_program():
    """Build the SPMD Bass program (identical on all 8 cores)."""
    nc = bacc.Bacc("TRN2", target_bir_lowering=False, debug=False,
                   num_devices=NCORES)

    # inputs pre-packed in TRANSFER UNITS: [:, u] is contiguous per
    # partition (8/4 KB DMA lines -> near-peak HBM rate per transfer)
    qT_d = nc.dram_tensor("qT", [P, 4, DCH, 512], BF,
                          kind="ExternalInput").ap()
    kT_d = nc.dram_tensor("kT", [P, 2, DCH, 512], BF,
                          kind="ExternalInput").ap()
    vT_d = nc.dram_tensor("vT", [P, 4, DCH, 256], BF,
                          kind="ExternalInput").ap()
    wall_d = nc.dram_tensor("wall", [P, DCH, 3 * H], BF,
                            kind="ExternalInput").ap()
    ball_d = nc.dram_tensor("ball", [H, 3], FP, kind="ExternalInput").ap()
    nmask = len(KS_MASKED)
    thr_d = nc.dram_tensor("thr", [P, nmask, 2], FP,
                           kind="ExternalInput").ap()
    out_d = nc.dram_tensor("out", [H + 1, NPAIRS, 2 * CHUNK], FP,
                           kind="ExternalOutput").ap()

    with tile.TileContext(nc) as tc:
        with (
            tc.tile_pool(name="const", bufs=1) as const,
            tc.tile_pool(name="resident", bufs=1) as res,
            tc.tile_pool(name="attp", bufs=26) as attp,
            tc.tile_pool(name="outp", bufs=2) as outp,
            tc.tile_pool(name="psc", bufs=4, space="PSUM") as psc,
            tc.tile_pool(name="pout", bufs=4, space="PSUM") as pout,
        ):
            # ---- constants on the scalar ring ----
            wall_sb = const.tile([P, DCH, 3 * H], BF, tag="wall")
            nc.scalar.dma_start(wall_sb, wall_d)
            ball_sb = const.tile([H, 3], FP, tag="ball")
            nc.scalar.dma_start(ball_sb, ball_d)
            thr_sb = const.tile([P, nmask, 2], FP, tag="thr")
            nc.scalar.dma_start(thr_sb, thr_d)
            wk_sb = wall_sb[:, :, 0:H]
            wv_sb = wall_sb[:, :, H:2 * H]
            wq_sb = wall_sb[:, :, 2 * H:3 * H]
            bk_sb = ball_sb[:, 0:1]
            bv_sb = ball_sb[:, 1:2]
            bq_sb = ball_sb[:, 2:3]
            zeros_sb = const.tile([P, 2 * CHUNK], BF, tag="zeros")
            nc.vector.memset(zeros_sb, 0.0)
            identb = const.tile([P, P], BF, tag="identb")
            make_identity(nc, identb)

            # ---- big input DMAs, ONE ring (sync), dependency order.
            # v is split in four 0.5 MB units slotted into the q/k
            # stream so av work fills PE gaps while q streams in, and
            # the post-DMA tail only depends on the last small v unit.
            xk_sb = res.tile([P, 2, DCH, 512], BF, tag="xk")
            xv_sb = res.tile([P, 4, DCH, 256], BF, tag="xv")
            xq_sb = res.tile([P, 4, DCH, 512], BF, tag="xq")

            def dma_unit(dst, src, u):
                nc.sync.dma_start(dst[:, u], src[:, u])

            dma_unit(xk_sb, kT_d, 0)
            dma_unit(xq_sb, qT_d, 0)
            dma_unit(xq_sb, qT_d, 1)
            dma_unit(xv_sb, vT_d, 0)
            dma_unit(xq_sb, qT_d, 2)
            dma_unit(xq_sb, qT_d, 3)
            dma_unit(xk_sb, kT_d, 1)
            dma_unit(xv_sb, vT_d, 1)
            dma_unit(xv_sb, vT_d, 2)
            dma_unit(xv_sb, vT_d, 3)

            # on-device predicates: pred[p, mi, h*256+f] = (f >= thr)
            pred_sb = res.tile([P, nmask, 2 * CHUNK], BF, tag="pred")
            iota_sb = const.tile([P, CHUNK], FP, tag="iota")
            nc.gpsimd.iota(iota_sb, pattern=[[1, CHUNK]], base=0,
                           channel_multiplier=0,
                           allow_small_or_imprecise_dtypes=True)
            for mi in range(nmask):
                for half in range(2):
                    nc.vector.tensor_scalar(
                        pred_sb[:, mi, ds(half * CHUNK, CHUNK)],
                        iota_sb, thr_sb[:, mi, ds(half, 1)], None,
                        mybir.AluOpType.is_ge)

            # ---- PE warm-up: keep HAM clock up until k0 lands ----
            WARM_MMS = 12
            pwarm = psc.tile([P, 2 * CHUNK], FP, tag="sc", name="pwarm")
            for _ in range(WARM_MMS):
                nc.tensor.matmul(pwarm, lhsT=identb,
                                 rhs=zeros_sb, start=True, stop=True)

            # col-group-paired projection: two M=64 matmuls concurrently
            # contract the same weight over two width/2-wide input chunks.
            # x3 is one transfer unit [P, DCH, width]; base is its global
            # column offset in the projected output.
            def proj_pair(w_sb, x3, base, width, out_fn, name):
                hw = width // 2
                pj = psc.tile([P, 2 * CHUNK], FP, tag="sc", name=name)
                for d in range(DCH):
                    nc.tensor.matmul(pj[0:H, 0:hw], lhsT=w_sb[:, d, :],
                                     rhs=x3[:, d, ds(0, hw)],
                                     start=(d == 0), stop=(d == DCH - 1),
                                     skip_group_check=True)
                    nc.tensor.matmul(pj[H:2 * H, 0:hw], lhsT=w_sb[:, d, :],
                                     rhs=x3[:, d, ds(hw, hw)],
                                     start=(d == 0), stop=(d == DCH - 1),
                                     tile_position=(0, H),
                                     skip_group_check=True)
                out_fn(pj[0:H, 0:hw], base)
                out_fn(pj[H:2 * H, 0:hw], base + hw)

            k_sb = res.tile([P, SL], BF, tag="k")
            nc.vector.memset(k_sb[H:, :], 0.0)
            vT_sb = res.tile([P, SL], BF, tag="vT")
            v_sb = res.tile([P, JTL, H + 1], BF, tag="v")
            # ones column (softmax denominator row) set once up front
            nc.vector.memset(v_sb[:, :, H:], 1.0)
            q_sb = res.tile([P, NQ], BF, tag="q")
            nc.vector.memset(q_sb[H:, :], 0.0)

            def k_out(pj, c0):
                nc.scalar.activation(k_sb[:H, ds(c0, pj.shape[-1])], pj,
                                     mybir.ActivationFunctionType.Identity,
                                     bias=bk_sb)

            def q_out(pj, c0):
                nc.scalar.activation(q_sb[:H, ds(c0, pj.shape[-1])], pj,
                                     mybir.ActivationFunctionType.Identity,
                                     bias=bq_sb)

            def v_out(pj, c0):
                n = pj.shape[-1]
                nc.scalar.activation(vT_sb[:H, ds(c0, n)], pj,
                                     mybir.ActivationFunctionType.Identity,
                                     bias=bv_sb)
                for jt in range(c0 // P, (c0 + n) // P):
                    pvt = psc.tile([P, P], BF, tag="sc", name="pvt")
                    nc.tensor.transpose(pvt, vT_sb[:, ds(jt * P, P)], identb)
                    nc.vector.tensor_copy(v_sb[:, jt, 0:H], pvt[:, :H])

            mask_idx = {sj: i for i, sj in enumerate(KS_MASKED)}
            W = 2 * CHUNK  # 512
            po_tiles = {}
            att_tiles = {}

            def emit_score(pr, jt):
                shared, solo = KS_PAIRS[pr]
                wide = jt < shared
                c0 = pr * W if wide else pr * W + CHUNK
                n = W if wide else CHUNK
                ps = psc.tile([P, n], FP, tag="sc", name="ps")
                nc.tensor.matmul(ps, lhsT=k_sb[:, ds(jt * P, P)],
                                 rhs=q_sb[:, ds(c0, n)],
                                 start=True, stop=True)
                att = attp.tile([P, n], BF, tag="att", name="att")
                # k is pre-scaled by 1/8 on host, so scores need no scale
                nc.scalar.activation(att, ps,
                                     mybir.ActivationFunctionType.Exp)
                mi = mask_idx.get((pr, jt))
                if mi is not None:
                    off = 0 if wide else CHUNK
                    nc.vector.tensor_mul(
                        att, att, pred_sb[:, mi, ds(off, n)])
                att_tiles[(pr, jt)] = (att, c0, n)

            def emit_av(pr, jt):
                solo = KS_PAIRS[pr][1]
                if pr not in po_tiles:
                    po_tiles[pr] = pout.tile([H + 1, W], FP, tag="po",
                                             name=f"po{pr}")
                att, c0, n = att_tiles.pop((pr, jt))
                nc.tensor.matmul(po_tiles[pr][:, ds(c0 - pr * W, n)],
                                 lhsT=v_sb[:, jt, :], rhs=att,
                                 start=(jt == 0), stop=(jt == solo - 1),
                                 skip_group_check=True)

            def epilogue(pr):
                po = po_tiles[pr]
                stage = outp.tile([H + 1, W], FP, tag="stage")
                nc.vector.tensor_copy(stage, po)
                nc.sync.dma_start(out_d[:, pr, :], stage)

            # ---- decoupled emission schedule (arrival order: k.u0,
            # q.u0, q.u1, v.u0, q.u2, q.u3, k.u1, v.u1, v.u2, v.u3) ----
            proj_pair(wk_sb, xk_sb[:, 0], 0, 512, k_out, "pk0")
            for _ in range(6):    # bridge PE idle until q.u0 arrives
                nc.tensor.matmul(pwarm, lhsT=identb, rhs=zeros_sb,
                                 start=True, stop=True)
            proj_pair(wq_sb, xq_sb[:, 0], 0, 512, q_out, "pq0a")
            emit_score(0, 0)
            emit_score(0, 1)
            proj_pair(wq_sb, xq_sb[:, 1], 512, 512, q_out, "pq0b")
            for jt in range(4):
                emit_score(1, jt)
            proj_pair(wv_sb, xv_sb[:, 0], 0, 256, v_out, "pv0a")
            emit_av(0, 0)
            emit_av(0, 1)
            epilogue(0)
            emit_av(1, 0)
            emit_av(1, 1)
            proj_pair(wq_sb, xq_sb[:, 2], 1024, 512, q_out, "pq1a")
            for jt in range(4):
                emit_score(2, jt)
            emit_av(2, 0)
            emit_av(2, 1)
            proj_pair(wq_sb, xq_sb[:, 3], 1536, 512, q_out, "pq1b")
            for jt in range(4):
                emit_score(3, jt)
            emit_av(3, 0)
            emit_av(3, 1)
            # The six k1-gated scores drip at exp pace (psc WAR); the
            # v-chain and data-ready avs are interleaved into that drip
            # so they fill the PE's exp-wait gaps instead of queueing
            # head-of-line behind all six scores.
            proj_pair(wk_sb, xk_sb[:, 1], 512, 512, k_out, "pk1")
            emit_score(2, 4)
            emit_score(2, 5)
            proj_pair(wv_sb, xv_sb[:, 1], 256, 256, v_out, "pv0b")
            emit_av(1, 2)
            emit_av(1, 3)
            epilogue(1)
            # materialize the remaining v tiles NOW (their data arrives
            # by here) so the final avs wait only on their exps, not on
            # a v-projection chain queued behind the late-score weave
            proj_pair(wv_sb, xv_sb[:, 2], 512, 256, v_out, "pv1a")
            proj_pair(wv_sb, xv_sb[:, 3], 768, 256, v_out, "pv1b")
            emit_score(3, 4)
            emit_score(3, 5)
            emit_av(2, 2)
            emit_av(2, 3)
            emit_score(3, 6)
            emit_score(3, 7)
            emit_av(3, 2)
            emit_av(3, 3)
            emit_av(2, 4)
            emit_av(2, 5)
            epilogue(2)
            emit_av(3, 4)
            emit_av(3, 5)
            emit_av(3, 6)
            emit_av(3, 7)
            epilogue(3)

    nc.compile()
    return nc


def _pack(xT):
    """[D, cols] -> [128, D/128, cols]: one contiguous DMA line/partition."""
    d, s = xT.shape
    return np.ascontiguousarray(
        xT.reshape(DCH, P, s).transpose(1, 0, 2)).astype(BF_NP)


def _np_reference(query, key, value, mask, Wq, bq, Wk, bk, Wv, bv):
    q = query @ Wq + bq
    k = key @ Wk + bk
    v = value @ Wv + bv
    scores = np.einsum("bqh,bkh->bqk", q, k) / np.sqrt(np.float32(H))
    scores = np.where(mask, scores, np.float32(-1e9))
    scores -= scores.max(axis=-1, keepdims=True)
    e = np.exp(scores)
    att = e / e.sum(axis=-1, keepdims=True)
    return np.einsum("bqk,bkh->bqh", att, v).astype(np.float32)


def kernel(query, key, value, mask, Wq, bq, Wk, bk, Wv, bv):
    global LAST_RESULTS
    query = np.asarray(query, dtype=np.float32)
    key = np.asarray(key, dtype=np.float32)
    value = np.asarray(value, dtype=np.float32)
    mask = np.asarray(mask).astype(bool)
    Wq = np.asarray(Wq, dtype=np.float32)
    Wk = np.asarray(Wk, dtype=np.float32)
    Wv = np.asarray(Wv, dtype=np.float32)
    bq = np.asarray(bq, dtype=np.float32)
    bk = np.asarray(bk, dtype=np.float32)
    bv = np.asarray(bv, dtype=np.float32)

    tril = np.tril(np.ones((S, S), dtype=bool))
    if not all(np.array_equal(mask[b], tril) for b in range(B)):
        # non-causal masks never occur for this problem; fall back to an
        # exact host implementation rather than an untested device path
        return _np_reference(query, key, value, mask, Wq, bq, Wk, bk,
                             Wv, bv)

    if "ks" not in _PROGRAM_CACHE:
        _PROGRAM_CACHE["ks"] = _build_program()
    nc = _PROGRAM_CACHE["ks"]

    def packw(w):
        return np.ascontiguousarray(
            w.reshape(DCH, P, H).transpose(1, 0, 2)).astype(BF_NP)

    # weight layout must match the wall_sb slicing: wk | wv | wq.
    # k is pre-scaled by 1/8 so scores come out of the matmul pre-scaled
    # and the exp activation needs no scale parameter.
    wall_in = np.concatenate(
        [packw(Wk * 0.125), packw(Wv), packw(Wq)], axis=2)
    wall_in = np.ascontiguousarray(wall_in)
    ball_in = np.ascontiguousarray(
        np.stack([bk * 0.125, bv, bq], axis=1).astype(np.float32))

    pvec = np.arange(P, dtype=np.float32)
    in_maps = []
    for c in range(NCORES):
        b, h = divmod(c, 2)
        jglob = JGLOB[h]
        # gather this core's key half in LOCAL tile order, then repack
        # into per-transfer units: [:, u] contiguous per partition
        cols = np.concatenate(
            [np.arange(j * P, (j + 1) * P) for j in jglob])

        def units(full, w):
            n = full.shape[-1] // w
            return np.ascontiguousarray(np.stack(
                [full[:, :, w * u:w * (u + 1)] for u in range(n)], axis=1))

        kT = units(_pack(key[b][cols].T), 512)      # [P, 2, DCH, 512]
        vT = units(_pack(value[b][cols].T), 256)    # [P, 4, DCH, 256]
        qT = units(_pack(query[b].T), 512)          # [P, 4, DCH, 512]
        # thresholds: pred[p, mi, half*256+f] = (f >= thr) allows keys
        # 128*Jglob[jt] + p <= 256*chunk + f
        thr = np.zeros((P, len(KS_MASKED), 2), dtype=np.float32)
        for mi, (pr, jt) in enumerate(KS_MASKED):
            for half in range(2):
                chunk = 2 * pr + half
                thr[:, mi, half] = jglob[jt] * P + pvec - chunk * CHUNK
        im = {"qT": qT, "kT": kT, "vT": vT,
              "wall": wall_in, "ball": ball_in,
              "thr": np.ascontiguousarray(thr)}
        in_maps.append(im)

    results = run_bass_kernel_spmd(
        nc, in_maps, core_ids=list(range(NCORES)),
        trace=bool(os.environ.get("BASS_TRACE")),
    )
    LAST_RESULTS = results

    out = np.empty((B, S, H), dtype=np.float32)
    for b in range(B):
        oA = results.results[2 * b]["out"].reshape(H + 1, NQ)
        oB = results.results[2 * b + 1]["out"].reshape(H + 1, NQ)
        num = oA[:H] + oB[:H]
        den = oA[H] + oB[H]
        out[b] = (num / den).T
    return out
